# revision 9
# baseline (speedup 1.0000x reference)
"""DeltaRNN (delta-threshold quantized LSTM) Trainium2 kernel.

Problem: T=512, B=64, I=512, H=1024, G=4*H=4096, fp32.

Sharding: 8-way tensor-parallel over the 4H gate dim. Each core owns 128
hidden units (gate columns ordered locally as i,f,o,g for contiguous
sigmoid). Batch (64) stays whole as the matmul M dim. The recurrence's only
cross-core data is the masked delta-h (dh after threshold masking), [128,64]
fp32 per core per step, exchanged with an AllGather.

Quantization: Q8.8 / Q2.8 fixed point reproduced exactly via fp32->int16
converts (hardware rounds RNE and saturates to [-32768,32767], which matches
jnp.round + the reference's clip bounds bit-exactly). Threshold masks use the
exact square trick: |d| >= th  <=>  d*d >= th*th (both sides exact dyadics).

Self-contained: includes the TileContext tail workaround for the local
walrus build (rejects InstDrain/multi-wait/eq-wait tail constructs).
"""
import re

import numpy as np

import concourse.bass as bass
import concourse.tile as tile
from concourse import mybir
from concourse.bass_utils import run_bass_kernel_spmd
from concourse.vector_clock import ScopedClock, VectorClock

F32 = mybir.dt.float32
I16 = mybir.dt.int16
A = mybir.AluOpType
ACT = mybir.ActivationFunctionType

T_FULL, B, I, H = 512, 64, 512, 1024
NCORES = 8
HK = H // NCORES           # 128 hidden units per core
GK = 4 * HK                # 512 gate columns per core
KI = I // 128              # 4 K-chunks for the x matmul
KH = H // 128              # 8 K-chunks for the h matmul
TH2 = float((26.0 / 256.0) ** 2)   # squared (quantized) threshold, exact
GATE_PERM = [0, 1, 3, 2]   # local block order (i,f,o,g) -> global gate idx


# ---------------------------------------------------------------------------
# TileContext tail workaround (local walrus rejects the stock drain/barrier)
# ---------------------------------------------------------------------------
_VC_RE = re.compile(r"VectorClock\((\[.*\])\)")
_MAX_WAITS = 1  # this walrus rejects instructions with >1 sync wait


def _split_excess_waits(mod):
    """Hoist sync waits beyond _MAX_WAITS onto NoOps inserted just before the
    instruction on the same engine (engine streams execute in block order)."""
    n_new = [0]

    def fix_block(block):
        insts = block.get("instructions")
        if not isinstance(insts, list):
            return
        out = []
        for ins in insts:
            si = ins.get("sync_info") or {}
            waits = si.get("on_wait") or []
            if len(waits) > _MAX_WAITS:
                extra = waits[:-_MAX_WAITS]
                si["on_wait"] = waits[-_MAX_WAITS:]
                for j in range(0, len(extra), _MAX_WAITS):
                    n_new[0] += 1
                    out.append({
                        "debug": ins.get("debug", 0),
                        "engine": ins["engine"],
                        "ins": [],
                        "name": f"{ins['name']}-w{j}",
                        "opcode": "NoOp",
                        "outs": [],
                        "sync_info": {"on_update": [],
                                      "on_wait": extra[j:j + _MAX_WAITS]},
                        "text_hint": "wait_split",
                    })
            out.append(ins)
        block["instructions"] = out

    def walk(obj):
        if isinstance(obj, dict):
            if "instructions" in obj:
                fix_block(obj)
            for v in obj.values():
                walk(v)
        elif isinstance(obj, list):
            for v in obj:
                walk(v)

    walk(mod)
    return n_new[0]


_orig_to_json_bytes = bass.Bass.to_json_bytes


def _patched_to_json_bytes(self, *a, **kw):
    import orjson
    raw = _orig_to_json_bytes(self, *a, **kw)
    mod = orjson.loads(raw)
    if _split_excess_waits(mod):
        raw = orjson.dumps(mod)
    return raw


bass.Bass.to_json_bytes = _patched_to_json_bytes


def _patched_drain_and_barrier(self, tick_clock, wait_clock):
    nc = self.nc
    vals = eval(_VC_RE.match(repr(tick_clock.global_clock)).group(1))  # noqa: S307
    for proc, tickv in enumerate(vals):
        if tickv <= 0:
            continue
        one = VectorClock()
        one.require_at_least(proc, tickv)
        nop_inst = nc.sync.nop(hint=f"tile_tail_wait_{proc}", nofuse=True)
        wait_clock.add_sem_waits(nop_inst.ins, ScopedClock({None: one}))
    nc.all_engine_barrier(sem_only=True)
    assert self.sems is not None
    popped = nc._tile_sem_poison_stack.pop()
    assert popped is self._sem_poison
    nc.clear_and_free_semaphores(list(self.sems.allocated().values()))
    nc.all_engine_barrier(sem_only=True)


tile.TileContext._drain_and_barrier = _patched_drain_and_barrier


# ---------------------------------------------------------------------------
# Device kernel
# ---------------------------------------------------------------------------
def build_kernel(T, exchange="collective"):
    nc = bass.Bass("TRN2", target_bir_lowering=False, num_devices=NCORES)

    xT = nc.dram_tensor("xT", [T, 128, KI * B], F32, kind="ExternalInput")
    wih = nc.dram_tensor("wih", [128, KI * GK], F32, kind="ExternalInput")
    whh = nc.dram_tensor("whh", [128, KH * GK], F32, kind="ExternalInput")
    bias = nc.dram_tensor("bias", [B, GK], F32, kind="ExternalInput")
    eye = nc.dram_tensor("eye", [B, B], F32, kind="ExternalInput")

    out_loc = nc.dram_tensor("out_loc", [T, B, HK], F32, kind="ExternalOutput")
    h_fin = nc.dram_tensor("h_fin", [B, HK], F32, kind="ExternalOutput")
    hp_fin = nc.dram_tensor("hp_fin", [B, HK], F32, kind="ExternalOutput")
    c_fin = nc.dram_tensor("c_fin", [B, HK], F32, kind="ExternalOutput")
    m_fin = nc.dram_tensor("m_fin", [B, GK], F32, kind="ExternalOutput")
    xp_fin = nc.dram_tensor("xp_fin", [128, KI * B], F32, kind="ExternalOutput")
    reg_fin = nc.dram_tensor("reg_fin", [B, 1], F32, kind="ExternalOutput")

    with tile.TileContext(nc) as tc:
        with (
            tc.tile_pool(name="const", bufs=1) as constp,
            tc.tile_pool(name="state", bufs=1) as statep,
            tc.tile_pool(name="xin", bufs=4) as xinp,
            tc.tile_pool(name="work", bufs=3) as workp,
            tc.tile_pool(name="gate", bufs=3) as gatep,
            tc.tile_pool(name="dhmx", bufs=2) as dhmxp,
            tc.tile_pool(name="psum", bufs=2, space="PSUM") as psump,
            tc.tile_pool(name="psumt", bufs=2, space="PSUM") as psumtp,
            tc.tile_pool(name="dram", bufs=2, space="DRAM") as dramp,
        ):
            # constants / weights
            wih_sb = constp.tile([128, KI * GK], F32)
            nc.sync.dma_start(wih_sb[:], wih[:])
            whh_sb = constp.tile([128, KH * GK], F32)
            nc.sync.dma_start(whh_sb[:], whh[:])
            eye_sb = constp.tile([B, B], F32)
            nc.sync.dma_start(eye_sb[:], eye[:])

            # persistent state
            m_sb = statep.tile([B, GK], F32)
            nc.sync.dma_start(m_sb[:], bias[:])
            xp = statep.tile([128, KI * B], F32)
            nc.vector.memset(xp[:], 0.0)
            hp = statep.tile([B, HK], F32)
            nc.vector.memset(hp[:], 0.0)
            c16 = statep.tile([B, HK], I16)
            nc.vector.memset(c16[:], 0)
            racc = statep.tile([B, 4], F32)
            nc.vector.memset(racc[:], 0.0)

            dhmT = None     # exchanged masked-dh, [128, KH*B], prev step's
            hn = None
            qc16 = None

            for t in range(T):
                last = t == T - 1

                # ---- dx chain (replicated on every core) ----
                xt_sb = xinp.tile([128, KI * B], F32, tag="xt")
                nc.sync.dma_start(xt_sb[:], xT[t])
                dxq = workp.tile([128, KI * B], F32, tag="dxq")
                nc.vector.tensor_tensor(dxq[:], xt_sb[:], xp[:], A.subtract)
                sqx = workp.tile([128, KI * B], F32, tag="sqx")
                nc.vector.tensor_tensor(sqx[:], dxq[:], dxq[:], A.mult)
                m01x = workp.tile([128, KI * B], F32, tag="m01x")
                nc.vector.tensor_scalar(m01x[:], sqx[:], TH2, None, A.is_ge)
                dxm = dhmxp.tile([128, KI * B], F32, tag="dxm")
                nc.vector.tensor_tensor(dxm[:], dxq[:], m01x[:], A.mult)
                nc.vector.tensor_tensor(xp[:], xp[:], dxm[:], A.add)

                # ---- matmuls accumulate dm = dx@Wih + dh@Whh in PSUM ----
                dm = psump.tile([B, GK], F32, tag="dm")
                for kc in range(KI):
                    nc.tensor.matmul(
                        dm[:], dxm[:, kc * B:(kc + 1) * B],
                        wih_sb[:, kc * GK:(kc + 1) * GK],
                        start=(kc == 0), stop=(t == 0 and kc == KI - 1))
                if t > 0:
                    # t=0 has dh == 0: no dh matmuls
                    for kc in range(KH):
                        nc.tensor.matmul(
                            dm[:], dhmT[:, kc * B:(kc + 1) * B],
                            whh_sb[:, kc * GK:(kc + 1) * GK],
                            start=False, stop=(kc == KH - 1))

                # m += dm ; pre-activation quantize Q8.8 -> int16 (RNE+sat)
                nc.vector.tensor_tensor(m_sb[:], m_sb[:], dm[:], A.add)
                pre16 = gatep.tile([B, GK], I16, tag="pre16")
                nc.vector.tensor_scalar(pre16[:], m_sb[:], 256.0, None, A.mult)

                # ---- gates: sigmoid(i,f,o) | tanh(g); quantize Q2.8 ----
                gates_f = gatep.tile([B, GK], F32, tag="gates_f")
                nc.scalar.activation(gates_f[:, 0:3 * HK], pre16[:, 0:3 * HK],
                                     ACT.Sigmoid, scale=1.0 / 256.0)
                nc.scalar.activation(gates_f[:, 3 * HK:GK], pre16[:, 3 * HK:GK],
                                     ACT.Tanh, scale=1.0 / 256.0)
                qg16 = gatep.tile([B, GK], I16, tag="qg16")
                nc.vector.tensor_scalar(qg16[:], gates_f[:], 256.0, None, A.mult)
                qi = qg16[:, 0:HK]
                qf = qg16[:, HK:2 * HK]
                qo = qg16[:, 2 * HK:3 * HK]
                qgg = qg16[:, 3 * HK:GK]

                # ---- c update: c = c*f + i*g (65536-scaled, exact ints) ----
                av = workp.tile([B, HK], F32, tag="av")
                nc.vector.tensor_tensor(av[:], c16[:], qf, A.mult)
                bv = workp.tile([B, HK], F32, tag="bv")
                nc.vector.tensor_tensor(bv[:], qi, qgg, A.mult)
                sv = workp.tile([B, HK], F32, tag="sv")
                nc.vector.tensor_tensor(sv[:], av[:], bv[:], A.add)
                qc16 = statep.tile([B, HK], I16, tag=f"qc{t % 2}")
                nc.vector.tensor_scalar(qc16[:], sv[:], 1.0 / 256.0, None, A.mult)

                # ---- h = quantize(o * quantize(tanh(c))) ----
                tc_f = workp.tile([B, HK], F32, tag="tc_f")
                nc.scalar.activation(tc_f[:], qc16[:], ACT.Tanh, scale=1.0 / 256.0)
                qt16 = workp.tile([B, HK], I16, tag="qt16")
                nc.vector.tensor_scalar(qt16[:], tc_f[:], 256.0, None, A.mult)
                hv = workp.tile([B, HK], F32, tag="hv")
                nc.vector.tensor_tensor(hv[:], qo, qt16[:], A.mult)
                h16 = workp.tile([B, HK], I16, tag="h16")
                nc.vector.tensor_scalar(h16[:], hv[:], 1.0 / 256.0, None, A.mult)
                hn = workp.tile([B, HK], F32, tag=f"hn{t % 2}")
                nc.vector.tensor_scalar(hn[:], h16[:], 1.0 / 256.0, None, A.mult)
                nc.sync.dma_start(out_loc[t], hn[:])
                c16 = qc16

                if last:
                    break

                # ---- masked delta-h ----
                dh = workp.tile([B, HK], F32, tag="dh")
                nc.vector.tensor_tensor(dh[:], hn[:], hp[:], A.subtract)
                sqh = workp.tile([B, HK], F32, tag="sqh")
                nc.vector.tensor_tensor(sqh[:], dh[:], dh[:], A.mult)
                m01h = workp.tile([B, HK], F32, tag="m01h")
                nc.vector.tensor_scalar(m01h[:], sqh[:], TH2, None, A.is_ge)
                dhm = dhmxp.tile([B, HK], F32, tag="dhm")
                nc.vector.tensor_tensor(dhm[:], dh[:], m01h[:], A.mult)
                nc.vector.tensor_tensor(hp[:], hp[:], dhm[:], A.add)

                # reg accumulation: racc[:,0] += sum(|dhm|)
                dabs = workp.tile([B, HK], F32, tag="dabs")
                rsum = workp.tile([B, 1], F32, tag="rsum")
                nc.scalar.activation(dabs[:], dhm[:], ACT.Abs, accum_out=rsum[:])
                nc.vector.tensor_tensor(racc[:, 0:1], racc[:, 0:1], rsum[:], A.add)

                # ---- transpose + all-gather the masked delta-h ----
                dhmT_ps = psumtp.tile([128, B], F32, tag="dhmT_ps")
                nc.tensor.transpose(dhmT_ps[:], dhm[:], eye_sb[:])
                dhmT_sb = workp.tile([128, B], F32, tag="dhmT_sb")
                nc.vector.tensor_copy(dhmT_sb[:], dhmT_ps[:])
                if exchange == "collective":
                    ag_in = dramp.tile([128, B], F32, tag="ag_in")
                    nc.sync.dma_start(ag_in[:], dhmT_sb[:])
                    ag_out = dramp.tile([NCORES * 128, B], F32, tag="ag_out",
                                        addr_space="Shared")
                    nc.gpsimd.collective_compute(
                        "AllGather",
                        A.bypass,
                        ins=[ag_in[:].opt()],
                        outs=[ag_out[:].opt()],
                        replica_groups=[list(range(NCORES))],
                    )
                    dhmT = dhmxp.tile([128, KH * B], F32, tag="dhmT")
                    nc.sync.dma_start(
                        dhmT[:], ag_out[:].rearrange("(c p) b -> p c b", p=128))
                else:  # timing-only variant: no cross-core exchange
                    dhmT = dhmxp.tile([128, KH * B], F32, tag="dhmT")
                    for kc in range(KH):
                        nc.vector.tensor_copy(dhmT[:, kc * B:(kc + 1) * B],
                                              dhmT_sb[:])

            # ---- final state outputs ----
            nc.sync.dma_start(h_fin[:], hn[:])
            nc.sync.dma_start(hp_fin[:], hp[:])
            cnat = workp.tile([B, HK], F32, tag="cnat")
            nc.vector.tensor_scalar(cnat[:], qc16[:], 1.0 / 256.0, None, A.mult)
            nc.sync.dma_start(c_fin[:], cnat[:])
            nc.sync.dma_start(m_fin[:], m_sb[:])
            nc.sync.dma_start(xp_fin[:], xp[:])
            nc.sync.dma_start(reg_fin[:], racc[:, 0:1])

    return nc


# ---------------------------------------------------------------------------
# Host side
# ---------------------------------------------------------------------------
def _prep_inputs(x, weight_ih, weight_hh, bias_ih, bias_hh, T):
    f32 = np.float32
    x = np.ascontiguousarray(x[:T], f32)
    # quantize x exactly like the reference (fp32 round-half-even + clip)
    xq = np.clip(np.round(x * f32(256.0)), -32768.0, 32767.0).astype(f32) / f32(256.0)
    # [T,B,I] -> [T, 128, KI*B] with element (t, p, kc*B+b) = xq[t, b, kc*128+p]
    xT = np.ascontiguousarray(
        xq.reshape(T, B, KI, 128).transpose(0, 3, 2, 1).reshape(T, 128, KI * B))

    eye = np.eye(B, dtype=f32)
    bsum = (bias_ih.astype(f32) + bias_hh.astype(f32)).astype(f32)

    in_maps = []
    for k in range(NCORES):
        cols = []
        for gl in GATE_PERM:
            g0 = gl * H + k * HK
            cols.extend(range(g0, g0 + HK))
        cols = np.array(cols)
        wih_k = weight_ih[cols, :].T.astype(f32)          # [I, GK]
        whh_k = weight_hh[cols, :].T.astype(f32)          # [H, GK]
        wih_sb = np.ascontiguousarray(
            wih_k.reshape(KI, 128, GK).transpose(1, 0, 2).reshape(128, KI * GK))
        whh_sb = np.ascontiguousarray(
            whh_k.reshape(KH, 128, GK).transpose(1, 0, 2).reshape(128, KH * GK))
        bias_k = np.ascontiguousarray(
            np.broadcast_to(bsum[cols][None, :], (B, GK)), dtype=f32)
        in_maps.append({
            "xT": xT, "wih": wih_sb, "whh": whh_sb, "bias": bias_k, "eye": eye,
        })
    return in_maps


def _assemble(results, T):
    f32 = np.float32
    out = np.zeros((T, B, H), f32)
    h = np.zeros((B, H), f32)
    hpv = np.zeros((B, H), f32)
    cv = np.zeros((B, H), f32)
    mv = np.zeros((B, 4 * H), f32)
    reg64 = 0.0
    for k, r in enumerate(results):
        sl = slice(k * HK, (k + 1) * HK)
        out[:, :, sl] = r["out_loc"]
        h[:, sl] = r["h_fin"]
        hpv[:, sl] = r["hp_fin"]
        cv[:, sl] = r["c_fin"]
        for bl, gl in enumerate(GATE_PERM):
            mv[:, gl * H + k * HK:gl * H + (k + 1) * HK] = \
                r["m_fin"][:, bl * HK:(bl + 1) * HK]
        reg64 += r["reg_fin"].astype(np.float64).sum()
    # x_prev: from core 0, [128, KI*B] -> [B, I], padded to max(I, H)
    xpT = results[0]["xp_fin"]
    xprev = xpT.reshape(128, KI, B).transpose(2, 1, 0).reshape(B, I)
    xpad = np.zeros((B, max(I, H)), f32)
    xpad[:, :I] = xprev
    reg = np.float32(reg64).reshape(1)
    return (out, xpad[None], h[None], hpv[None], cv[None], mv[None], reg)


def _run(x, weight_ih, weight_hh, bias_ih, bias_hh, T, **run_kwargs):
    in_maps = _prep_inputs(x, weight_ih, weight_hh, bias_ih, bias_hh, T)
    nc = build_kernel(T)
    res = run_bass_kernel_spmd(nc, in_maps, core_ids=list(range(NCORES)),
                               **run_kwargs)
    return _assemble(res.results, T), res


def kernel(x, weight_ih, weight_hh, bias_ih, bias_hh):
    outs, _ = _run(np.asarray(x), np.asarray(weight_ih), np.asarray(weight_hh),
                   np.asarray(bias_ih), np.asarray(bias_hh), T_FULL)
    return outs


# revision 10
# speedup vs baseline: 1.4030x; 1.4030x over previous
"""DeltaRNN (delta-threshold quantized LSTM) Trainium2 kernel.

Problem: T=512, B=64, I=512, H=1024, G=4*H=4096, fp32.

Sharding: 8-way tensor-parallel over the 4H gate dim. Each core owns 128
hidden units (gate columns ordered locally as i,f,o,g for contiguous
sigmoid). Batch (64) stays whole as the matmul M dim. The recurrence's only
cross-core data is the masked delta-h (dh after threshold masking), [128,64]
fp32 per core per step, exchanged with an AllGather.

Quantization: Q8.8 / Q2.8 fixed point reproduced exactly via fp32->int16
converts (hardware rounds RNE and saturates to [-32768,32767], which matches
jnp.round + the reference's clip bounds bit-exactly). Threshold masks use the
exact square trick: |d| >= th  <=>  d*d >= th*th (both sides exact dyadics).

Self-contained: includes the TileContext tail workaround for the local
walrus build (rejects InstDrain/multi-wait/eq-wait tail constructs).
"""
import re

import numpy as np

import concourse.bass as bass
import concourse.tile as tile
from concourse import mybir
from concourse.bass_utils import run_bass_kernel_spmd
from concourse.vector_clock import ScopedClock, VectorClock

F32 = mybir.dt.float32
I16 = mybir.dt.int16
A = mybir.AluOpType
ACT = mybir.ActivationFunctionType

T_FULL, B, I, H = 512, 64, 512, 1024
NCORES = 8
HK = H // NCORES           # 128 hidden units per core
GK = 4 * HK                # 512 gate columns per core
KI = I // 128              # 4 K-chunks for the x matmul
KH = H // 128              # 8 K-chunks for the h matmul
TH2 = float((26.0 / 256.0) ** 2)   # squared (quantized) threshold, exact
GATE_PERM = [0, 1, 3, 2]   # local block order (i,f,o,g) -> global gate idx


# ---------------------------------------------------------------------------
# TileContext tail workaround (local walrus rejects the stock drain/barrier)
# ---------------------------------------------------------------------------
_VC_RE = re.compile(r"VectorClock\((\[.*\])\)")
_MAX_WAITS = 1  # this walrus rejects instructions with >1 sync wait


def _split_excess_waits(mod):
    """Hoist sync waits beyond _MAX_WAITS onto NoOps inserted just before the
    instruction on the same engine (engine streams execute in block order)."""
    n_new = [0]

    def fix_block(block):
        insts = block.get("instructions")
        if not isinstance(insts, list):
            return
        out = []
        for ins in insts:
            si = ins.get("sync_info") or {}
            waits = si.get("on_wait") or []
            if len(waits) > _MAX_WAITS:
                extra = waits[:-_MAX_WAITS]
                si["on_wait"] = waits[-_MAX_WAITS:]
                for j in range(0, len(extra), _MAX_WAITS):
                    n_new[0] += 1
                    out.append({
                        "debug": ins.get("debug", 0),
                        "engine": ins["engine"],
                        "ins": [],
                        "name": f"{ins['name']}-w{j}",
                        "opcode": "NoOp",
                        "outs": [],
                        "sync_info": {"on_update": [],
                                      "on_wait": extra[j:j + _MAX_WAITS]},
                        "text_hint": "wait_split",
                    })
            out.append(ins)
        block["instructions"] = out

    def walk(obj):
        if isinstance(obj, dict):
            if "instructions" in obj:
                fix_block(obj)
            for v in obj.values():
                walk(v)
        elif isinstance(obj, list):
            for v in obj:
                walk(v)

    walk(mod)
    return n_new[0]


_orig_to_json_bytes = bass.Bass.to_json_bytes


def _patched_to_json_bytes(self, *a, **kw):
    import orjson
    raw = _orig_to_json_bytes(self, *a, **kw)
    mod = orjson.loads(raw)
    if _split_excess_waits(mod):
        raw = orjson.dumps(mod)
    return raw


bass.Bass.to_json_bytes = _patched_to_json_bytes


def _patched_drain_and_barrier(self, tick_clock, wait_clock):
    nc = self.nc
    vals = eval(_VC_RE.match(repr(tick_clock.global_clock)).group(1))  # noqa: S307
    for proc, tickv in enumerate(vals):
        if tickv <= 0:
            continue
        one = VectorClock()
        one.require_at_least(proc, tickv)
        nop_inst = nc.sync.nop(hint=f"tile_tail_wait_{proc}", nofuse=True)
        wait_clock.add_sem_waits(nop_inst.ins, ScopedClock({None: one}))
    nc.all_engine_barrier(sem_only=True)
    assert self.sems is not None
    popped = nc._tile_sem_poison_stack.pop()
    assert popped is self._sem_poison
    nc.clear_and_free_semaphores(list(self.sems.allocated().values()))
    nc.all_engine_barrier(sem_only=True)


tile.TileContext._drain_and_barrier = _patched_drain_and_barrier


# ---------------------------------------------------------------------------
# Device kernel
# ---------------------------------------------------------------------------
def build_kernel(T, exchange="collective"):
    nc = bass.Bass("TRN2", target_bir_lowering=False, num_devices=NCORES)

    xT = nc.dram_tensor("xT", [T, 128, KI * B], F32, kind="ExternalInput")
    wih = nc.dram_tensor("wih", [128, KI * GK], F32, kind="ExternalInput")
    whh = nc.dram_tensor("whh", [128, KH * GK], F32, kind="ExternalInput")
    bias = nc.dram_tensor("bias", [B, GK], F32, kind="ExternalInput")
    eye = nc.dram_tensor("eye", [B, B], F32, kind="ExternalInput")

    out_loc = nc.dram_tensor("out_loc", [T, B, HK], F32, kind="ExternalOutput")
    h_fin = nc.dram_tensor("h_fin", [B, HK], F32, kind="ExternalOutput")
    hp_fin = nc.dram_tensor("hp_fin", [B, HK], F32, kind="ExternalOutput")
    c_fin = nc.dram_tensor("c_fin", [B, HK], F32, kind="ExternalOutput")
    m_fin = nc.dram_tensor("m_fin", [B, GK], F32, kind="ExternalOutput")
    xp_fin = nc.dram_tensor("xp_fin", [128, KI * B], F32, kind="ExternalOutput")
    reg_fin = nc.dram_tensor("reg_fin", [B, 1], F32, kind="ExternalOutput")

    with tile.TileContext(nc) as tc:
        with (
            tc.tile_pool(name="const", bufs=1) as constp,
            tc.tile_pool(name="state", bufs=1) as statep,
            tc.tile_pool(name="xin", bufs=4) as xinp,
            tc.tile_pool(name="work", bufs=3) as workp,
            tc.tile_pool(name="gate", bufs=3) as gatep,
            tc.tile_pool(name="dhmx", bufs=2) as dhmxp,
            tc.tile_pool(name="psum", bufs=2, space="PSUM") as psump,
            tc.tile_pool(name="psumt", bufs=2, space="PSUM") as psumtp,
            tc.tile_pool(name="dram", bufs=2, space="DRAM") as dramp,
        ):
            # constants / weights
            wih_sb = constp.tile([128, KI * GK], F32)
            nc.sync.dma_start(wih_sb[:], wih[:])
            whh_sb = constp.tile([128, KH * GK], F32)
            nc.sync.dma_start(whh_sb[:], whh[:])
            eye_sb = constp.tile([B, B], F32)
            nc.sync.dma_start(eye_sb[:], eye[:])

            # persistent state
            m_sb = statep.tile([B, GK], F32)
            nc.sync.dma_start(m_sb[:], bias[:])
            xp = statep.tile([128, KI * B], F32)
            nc.vector.memset(xp[:], 0.0)
            hp = statep.tile([B, HK], F32)
            nc.vector.memset(hp[:], 0.0)
            c16 = statep.tile([B, HK], I16)
            nc.vector.memset(c16[:], 0)
            racc = statep.tile([B, 4], F32)
            nc.vector.memset(racc[:], 0.0)

            dhmT = None     # exchanged masked-dh, [128, KH*B], prev step's
            hn = None
            qc16 = None

            for t in range(T):
                last = t == T - 1

                # ---- dx chain (replicated on every core) ----
                xt_sb = xinp.tile([128, KI * B], F32, tag="xt")
                nc.sync.dma_start(xt_sb[:], xT[t])
                # dx chain on GPSIMD: keeps these 5 ops off the busy DVE
                dxq = workp.tile([128, KI * B], F32, tag="dxq")
                nc.gpsimd.tensor_tensor(dxq[:], xt_sb[:], xp[:], A.subtract)
                sqx = workp.tile([128, KI * B], F32, tag="sqx")
                nc.gpsimd.tensor_tensor(sqx[:], dxq[:], dxq[:], A.mult)
                m01x = workp.tile([128, KI * B], F32, tag="m01x")
                nc.gpsimd.tensor_scalar(m01x[:], sqx[:], TH2, None, A.is_ge)
                dxm = dhmxp.tile([128, KI * B], F32, tag="dxm")
                nc.gpsimd.tensor_tensor(dxm[:], dxq[:], m01x[:], A.mult)
                nc.gpsimd.tensor_tensor(xp[:], xp[:], dxm[:], A.add)

                # ---- matmuls accumulate dm = dx@Wih + dh@Whh in PSUM ----
                dm = psump.tile([B, GK], F32, tag="dm")
                for kc in range(KI):
                    nc.tensor.matmul(
                        dm[:], dxm[:, kc * B:(kc + 1) * B],
                        wih_sb[:, kc * GK:(kc + 1) * GK],
                        start=(kc == 0), stop=(t == 0 and kc == KI - 1))
                if t > 0:
                    # t=0 has dh == 0: no dh matmuls
                    for kc in range(KH):
                        nc.tensor.matmul(
                            dm[:], dhmT[:, kc * B:(kc + 1) * B],
                            whh_sb[:, kc * GK:(kc + 1) * GK],
                            start=False, stop=(kc == KH - 1))

                # m += dm ; pre-activation quantize Q8.8 -> int16 (RNE+sat)
                nc.vector.tensor_tensor(m_sb[:], m_sb[:], dm[:], A.add)
                pre16 = gatep.tile([B, GK], I16, tag="pre16")
                nc.vector.tensor_scalar(pre16[:], m_sb[:], 256.0, None, A.mult)

                # ---- gates: sigmoid(i,f,o) | tanh(g); quantize Q2.8 ----
                gates_f = gatep.tile([B, GK], F32, tag="gates_f")
                nc.scalar.activation(gates_f[:, 0:3 * HK], pre16[:, 0:3 * HK],
                                     ACT.Sigmoid, scale=1.0 / 256.0)
                nc.scalar.activation(gates_f[:, 3 * HK:GK], pre16[:, 3 * HK:GK],
                                     ACT.Tanh, scale=1.0 / 256.0)
                qg16 = gatep.tile([B, GK], I16, tag="qg16")
                nc.vector.tensor_scalar(qg16[:], gates_f[:], 256.0, None, A.mult)
                qi = qg16[:, 0:HK]
                qf = qg16[:, HK:2 * HK]
                qo = qg16[:, 2 * HK:3 * HK]
                qgg = qg16[:, 3 * HK:GK]

                # ---- c update: c = c*f + i*g (65536-scaled, exact ints) ----
                av = workp.tile([B, HK], F32, tag="av")
                nc.vector.tensor_tensor(av[:], c16[:], qf, A.mult)
                bv = workp.tile([B, HK], F32, tag="bv")
                nc.vector.tensor_tensor(bv[:], qi, qgg, A.mult)
                sv = workp.tile([B, HK], F32, tag="sv")
                nc.vector.tensor_tensor(sv[:], av[:], bv[:], A.add)
                qc16 = statep.tile([B, HK], I16, tag=f"qc{t % 2}")
                nc.vector.tensor_scalar(qc16[:], sv[:], 1.0 / 256.0, None, A.mult)

                # ---- h = quantize(o * quantize(tanh(c))) ----
                tc_f = workp.tile([B, HK], F32, tag="tc_f")
                nc.scalar.activation(tc_f[:], qc16[:], ACT.Tanh, scale=1.0 / 256.0)
                qt16 = workp.tile([B, HK], I16, tag="qt16")
                nc.vector.tensor_scalar(qt16[:], tc_f[:], 256.0, None, A.mult)
                hv = workp.tile([B, HK], F32, tag="hv")
                nc.vector.tensor_tensor(hv[:], qo, qt16[:], A.mult)
                h16 = workp.tile([B, HK], I16, tag="h16")
                nc.vector.tensor_scalar(h16[:], hv[:], 1.0 / 256.0, None, A.mult)
                hn = workp.tile([B, HK], F32, tag=f"hn{t % 2}")
                nc.vector.tensor_scalar(hn[:], h16[:], 1.0 / 256.0, None, A.mult)
                nc.sync.dma_start(out_loc[t], hn[:])
                c16 = qc16

                if last:
                    break

                # ---- masked delta-h ----
                dh = workp.tile([B, HK], F32, tag="dh")
                nc.vector.tensor_tensor(dh[:], hn[:], hp[:], A.subtract)
                sqh = workp.tile([B, HK], F32, tag="sqh")
                nc.vector.tensor_tensor(sqh[:], dh[:], dh[:], A.mult)
                m01h = workp.tile([B, HK], F32, tag="m01h")
                nc.vector.tensor_scalar(m01h[:], sqh[:], TH2, None, A.is_ge)
                dhm = dhmxp.tile([B, HK], F32, tag="dhm")
                nc.vector.tensor_tensor(dhm[:], dh[:], m01h[:], A.mult)
                nc.vector.tensor_tensor(hp[:], hp[:], dhm[:], A.add)

                # reg accumulation: racc[:,0] += sum(|dhm|)
                dabs = workp.tile([B, HK], F32, tag="dabs")
                rsum = workp.tile([B, 1], F32, tag="rsum")
                nc.scalar.activation(dabs[:], dhm[:], ACT.Abs, accum_out=rsum[:])
                nc.vector.tensor_tensor(racc[:, 0:1], racc[:, 0:1], rsum[:], A.add)

                # ---- transpose + all-gather the masked delta-h ----
                dhmT_ps = psumtp.tile([128, B], F32, tag="dhmT_ps")
                nc.tensor.transpose(dhmT_ps[:], dhm[:], eye_sb[:])
                dhmT_sb = workp.tile([128, B], F32, tag="dhmT_sb")
                nc.vector.tensor_copy(dhmT_sb[:], dhmT_ps[:])
                if exchange == "collective":
                    ag_in = dramp.tile([128, B], F32, tag="ag_in")
                    nc.sync.dma_start(ag_in[:], dhmT_sb[:])
                    ag_out = dramp.tile([NCORES * 128, B], F32, tag="ag_out",
                                        addr_space="Shared")
                    nc.gpsimd.collective_compute(
                        "AllGather",
                        A.bypass,
                        ins=[ag_in[:].opt()],
                        outs=[ag_out[:].opt()],
                        replica_groups=[list(range(NCORES))],
                    )
                    dhmT = dhmxp.tile([128, KH * B], F32, tag="dhmT")
                    nc.sync.dma_start(
                        dhmT[:], ag_out[:].rearrange("(c p) b -> p c b", p=128))
                else:  # timing-only variant: no cross-core exchange
                    dhmT = dhmxp.tile([128, KH * B], F32, tag="dhmT")
                    for kc in range(KH):
                        nc.vector.tensor_copy(dhmT[:, kc * B:(kc + 1) * B],
                                              dhmT_sb[:])

            # ---- final state outputs ----
            nc.sync.dma_start(h_fin[:], hn[:])
            nc.sync.dma_start(hp_fin[:], hp[:])
            cnat = workp.tile([B, HK], F32, tag="cnat")
            nc.vector.tensor_scalar(cnat[:], qc16[:], 1.0 / 256.0, None, A.mult)
            nc.sync.dma_start(c_fin[:], cnat[:])
            nc.sync.dma_start(m_fin[:], m_sb[:])
            nc.sync.dma_start(xp_fin[:], xp[:])
            nc.sync.dma_start(reg_fin[:], racc[:, 0:1])

    return nc


# ---------------------------------------------------------------------------
# Host side
# ---------------------------------------------------------------------------
def _prep_inputs(x, weight_ih, weight_hh, bias_ih, bias_hh, T):
    f32 = np.float32
    x = np.ascontiguousarray(x[:T], f32)
    # quantize x exactly like the reference (fp32 round-half-even + clip)
    xq = np.clip(np.round(x * f32(256.0)), -32768.0, 32767.0).astype(f32) / f32(256.0)
    # [T,B,I] -> [T, 128, KI*B] with element (t, p, kc*B+b) = xq[t, b, kc*128+p]
    xT = np.ascontiguousarray(
        xq.reshape(T, B, KI, 128).transpose(0, 3, 2, 1).reshape(T, 128, KI * B))

    eye = np.eye(B, dtype=f32)
    bsum = (bias_ih.astype(f32) + bias_hh.astype(f32)).astype(f32)

    in_maps = []
    for k in range(NCORES):
        cols = []
        for gl in GATE_PERM:
            g0 = gl * H + k * HK
            cols.extend(range(g0, g0 + HK))
        cols = np.array(cols)
        wih_k = weight_ih[cols, :].T.astype(f32)          # [I, GK]
        whh_k = weight_hh[cols, :].T.astype(f32)          # [H, GK]
        wih_sb = np.ascontiguousarray(
            wih_k.reshape(KI, 128, GK).transpose(1, 0, 2).reshape(128, KI * GK))
        whh_sb = np.ascontiguousarray(
            whh_k.reshape(KH, 128, GK).transpose(1, 0, 2).reshape(128, KH * GK))
        bias_k = np.ascontiguousarray(
            np.broadcast_to(bsum[cols][None, :], (B, GK)), dtype=f32)
        in_maps.append({
            "xT": xT, "wih": wih_sb, "whh": whh_sb, "bias": bias_k, "eye": eye,
        })
    return in_maps


def _assemble(results, T):
    f32 = np.float32
    out = np.zeros((T, B, H), f32)
    h = np.zeros((B, H), f32)
    hpv = np.zeros((B, H), f32)
    cv = np.zeros((B, H), f32)
    mv = np.zeros((B, 4 * H), f32)
    reg64 = 0.0
    for k, r in enumerate(results):
        sl = slice(k * HK, (k + 1) * HK)
        out[:, :, sl] = r["out_loc"]
        h[:, sl] = r["h_fin"]
        hpv[:, sl] = r["hp_fin"]
        cv[:, sl] = r["c_fin"]
        for bl, gl in enumerate(GATE_PERM):
            mv[:, gl * H + k * HK:gl * H + (k + 1) * HK] = \
                r["m_fin"][:, bl * HK:(bl + 1) * HK]
        reg64 += r["reg_fin"].astype(np.float64).sum()
    # x_prev: from core 0, [128, KI*B] -> [B, I], padded to max(I, H)
    xpT = results[0]["xp_fin"]
    xprev = xpT.reshape(128, KI, B).transpose(2, 1, 0).reshape(B, I)
    xpad = np.zeros((B, max(I, H)), f32)
    xpad[:, :I] = xprev
    reg = np.float32(reg64).reshape(1)
    return (out, xpad[None], h[None], hpv[None], cv[None], mv[None], reg)


def _run(x, weight_ih, weight_hh, bias_ih, bias_hh, T, **run_kwargs):
    in_maps = _prep_inputs(x, weight_ih, weight_hh, bias_ih, bias_hh, T)
    nc = build_kernel(T)
    res = run_bass_kernel_spmd(nc, in_maps, core_ids=list(range(NCORES)),
                               **run_kwargs)
    return _assemble(res.results, T), res


def kernel(x, weight_ih, weight_hh, bias_ih, bias_hh):
    outs, _ = _run(np.asarray(x), np.asarray(weight_ih), np.asarray(weight_hh),
                   np.asarray(bias_ih), np.asarray(bias_hh), T_FULL)
    return outs
